# revision 6
# baseline (speedup 1.0000x reference)
"""Bass/Trainium2 kernel for GQA transformer block (nn_GQA_84353157694016).

Reference computation (B=2, S=2048, E=4096, H=32 q-heads, KVH=8 kv-heads, D=128):
    qkv = x @ wqkv.T                  -> split into q/k/v per GQA group
    q,k = rope_interleaved(q), rope_interleaved(k)
    out = softmax(causal(q k^T / sqrt(D))) @ v @ wo.T

Sharding (8 cores): data-parallel over batch (2 groups of 4 cores) x
tensor-parallel over heads (4 cores: 2 kv groups / 8 q heads each).  wo is
sharded on its input dim; the partial outputs are summed on the host
(the unshard step of the reduce).

Layout strategy: everything on-chip is computed in "transposed" (feature x
sequence) orientation so the TensorE contraction dim always lands on
partitions with zero on-chip transposes (except v, which is PE-transposed).
Softmax is computed without max-subtraction (scores are bounded ~ +-10 for
this problem size/scale).

Scheduling strategy (v1):
  - score tiles are produced in PAIRS into [128,1024] 2-bank PSUM tiles so
    each exp ACTIVATE covers 1024 columns (amortizes the 352-cycle ACT
    instruction overhead, the attention-phase co-bottleneck).
  - diagonal (causally trimmed) tiles skip the masked columns in the QK
    matmul, the AV matmul, and the row-sum accumulation (no zero-fill
    memsets; the garbage columns of the pair are simply never read).
  - row sums: two bf16 accumulators (accL on DVE, accR on GpSimd) fold all
    score tiles of a head, then a single 2-matmul ones-reduction per head.
  - output is written bf16, one batched DMA per (strip, e-chunk) block,
    issued from the GpSimd queue so the Scalar queue never blocks exp.
  - startup: the first chunk's wqkv/x loads are spread across 4 engine
    queues so the first matmul starts ~10us earlier.
"""

import os
import sys

import numpy as np
import ml_dtypes

for _p in ("/opt/trn_rl_repo",):
    if _p not in sys.path and os.path.isdir(_p):
        sys.path.append(_p)

import concourse.bass as bass
import concourse.tile as tile
from concourse import bacc, mybir
from concourse.bass_utils import run_bass_kernel_spmd
from concourse.masks import make_identity


def _install_ntff_hook():
    """bass_utils' trace path imports antenv.axon_hooks, which the agent image
    lacks; synthesize it (backed by trn_boot's ctypes NTFF driver) so
    trace=True / BASS_TRACE=1 works instead of crashing."""
    try:
        import antenv.axon_hooks  # noqa: F401
        return
    except ImportError:
        pass
    try:
        import types
        import antenv
        mod = types.ModuleType("antenv.axon_hooks")
        mod._hook = None
        mod.set_axon_ntff_profile_hook = lambda h: setattr(mod, "_hook", h)
        mod.get_axon_ntff_profile_hook = lambda: mod._hook
        sys.modules["antenv.axon_hooks"] = mod
        antenv.axon_hooks = mod
        from trn_agent_boot.trn_boot import _ntff_profile_via_ctypes
        so = "/opt/axon/libaxon_pjrt.so"
        if os.path.exists(so):
            mod._hook = _ntff_profile_via_ctypes(so)
    except Exception:
        pass


_install_ntff_hook()

# problem constants
B, S, E = 2, 2048, 4096
H, KVH, D = 32, 8, 128
QPK = H // KVH                    # 4 q heads per kv group
ROPE_BASE = 10000.0

NCORES = 8
TP = 4                            # tensor-parallel width (heads)
DP = 2                            # data-parallel width (batch)

SC = 4                            # number of s-chunks == q strips
CW = S // SC                      # 512 chunk width
NJT = (E + 2 * KVH * D) // TP // 128   # 12 qkv row-tiles per core
NET = E // 128                    # 32 contraction tiles for qkv proj
GPC = KVH // TP                   # 2 kv groups per core
HPC = H // TP                     # 8 q heads per core
FT = HPC * D // 128               # 8 local ctx feature tiles
ECN = E // CW                     # 8 output e-chunks

f32 = mybir.dt.float32
bf16 = mybir.dt.bfloat16
np_bf16 = ml_dtypes.bfloat16

_built = {}


def _build_nc():
    nc = bacc.Bacc("TRN2", target_bir_lowering=False)

    xt_d = nc.dram_tensor("xt", [SC, 128, NET, CW], bf16, kind="ExternalInput")
    wq_d = nc.dram_tensor("wq", [NJT, 128, NET, 128], bf16, kind="ExternalInput")
    wo_d = nc.dram_tensor("wo", [ECN, 128, FT, CW], bf16, kind="ExternalInput")
    cq_d = nc.dram_tensor("cq", [128, S], f32, kind="ExternalInput")
    sq_d = nc.dram_tensor("sq", [128, S], f32, kind="ExternalInput")
    ck_d = nc.dram_tensor("ck", [128, S], f32, kind="ExternalInput")
    sk_d = nc.dram_tensor("sk", [128, S], f32, kind="ExternalInput")
    mk_d = nc.dram_tensor("mk", [128, SC, CW], bf16, kind="ExternalInput")
    out_d = nc.dram_tensor("out", [SC, ECN, 128, CW // 128, CW], bf16,
                           kind="ExternalOutput")

    from contextlib import ExitStack
    with tile.TileContext(nc) as tc:
        with ExitStack() as _stk:
            def _pool(*a, **kw):
                return _stk.enter_context(tc.tile_pool(*a, **kw))
            constp = _pool(name="const", bufs=1)
            tabp = _pool(name="tab", bufs=2)
            xtp = _pool(name="xt", bufs=3)
            wqp = _pool(name="wq", bufs=2)
            stp = _pool(name="st", bufs=8)
            rtp = _pool(name="rt", bufs=2)
            qp = _pool(name="q", bufs=2)
            kvp = _pool(name="kv", bufs=1)
            atp = _pool(name="at", bufs=4)
            accp = _pool(name="acc", bufs=2)
            ctxp = _pool(name="ctx", bufs=2)
            wop = _pool(name="wop", bufs=4)
            obp = _pool(name="ob", bufs=1)
            rcp = _pool(name="rc", bufs=1)
            pmm = _pool(name="pmm", bufs=2, space="PSUM")
            pqk = _pool(name="pqk", bufs=2, space="PSUM")
            pacc = _pool(name="pacc", bufs=2, space="PSUM")
            def emit_wo_block(cs, ec, ctx_tiles, drain=False):
                """Output-projection block: out[strip cs, ec] += ctx @ woT."""
                wo_sb = wop.tile([128, FT, CW], bf16, tag="wo", name="wo_sb")
                if drain:
                    _e = (nc.sync, nc.scalar, nc.gpsimd)
                    engs = (_e[(2 * ec) % 3], _e[(2 * ec + 1) % 3])
                else:
                    engs = (nc.scalar, nc.gpsimd)
                engs[0].dma_start(out=wo_sb[:, :FT // 2, :],
                                  in_=wo_d[ec, :, :FT // 2, :])
                engs[1].dma_start(out=wo_sb[:, FT // 2:, :],
                                  in_=wo_d[ec, :, FT // 2:, :])
                ob = obp.tile([128, CW // 128, CW], bf16, tag="ob", name="ob")
                for sti in range(CW // 128):
                    ps = pacc.tile([128, CW], f32, tag="acc", name="wo_ps")
                    for ft in range(FT):
                        nc.tensor.matmul(
                            ps,
                            lhsT=ctx_tiles[:, ft, sti * 128:(sti + 1) * 128],
                            rhs=wo_sb[:, ft, :],
                            start=(ft == 0),
                            stop=(ft == FT - 1),
                        )
                    nc.vector.tensor_copy(ob[:, sti, :], ps)
                nc.gpsimd.dma_start(out=out_d[cs, ec], in_=ob)

            # constants
            ident = constp.tile([128, 128], f32, tag="ident")
            make_identity(nc, ident)
            ones_sb = constp.tile([128, 128], bf16, tag="ones")
            nc.vector.memset(ones_sb, 1.0)
            mk_sb = constp.tile([128, SC, CW], bf16, tag="mk")

            # persistent k (transposed) and v (natural) per kv group, bf16
            k_sb = [kvp.tile([128, S], bf16, tag=f"k{g}", name=f"k{g}")
                    for g in range(GPC)]
            v_sb = [kvp.tile([128, S // 128, 128], bf16, tag=f"v{g}", name=f"v{g}")
                    for g in range(GPC)]

            for c in range(SC):
                csl = slice(c * CW, (c + 1) * CW)
                # startup-ordered input loads; chunk 0 spreads the four big
                # transfers across four engine queues so they run in parallel
                wq_pre = []
                xt_h = []
                ld_eng = ((nc.sync, nc.scalar, nc.sync, nc.gpsimd)
                          if c == 0 else
                          (nc.sync, nc.sync, nc.sync, nc.sync))
                for pre in range(2):
                    w_ = wqp.tile([128, NET, 128], bf16, tag="wq", name="wq_pre")
                    ld_eng[2 * pre].dma_start(out=w_, in_=wq_d[pre])
                    wq_pre.append(w_)
                    xh = xtp.tile([128, NET // 2, CW], bf16, tag="xt", name="xh")
                    ld_eng[2 * pre + 1].dma_start(
                        out=xh,
                        in_=xt_d[c, :, pre * (NET // 2):(pre + 1) * (NET // 2), :],
                    )
                    xt_h.append(xh)

                # per-chunk rope table slices (needed only at RoPE time)
                cq_sb = tabp.tile([128, CW], f32, tag="cq")
                sq_sb = tabp.tile([128, CW], f32, tag="sq")
                ck_sb = tabp.tile([128, CW], f32, tag="ck")
                sk_sb = tabp.tile([128, CW], f32, tag="sk")
                nc.gpsimd.dma_start(out=cq_sb, in_=cq_d[:, csl])
                nc.gpsimd.dma_start(out=sq_sb, in_=sq_d[:, csl])
                nc.gpsimd.dma_start(out=ck_sb, in_=ck_d[:, csl])
                nc.gpsimd.dma_start(out=sk_sb, in_=sk_d[:, csl])
                if c == 0:
                    nc.gpsimd.dma_start(out=mk_sb, in_=mk_d[:])

                # ---- fused QKV projection + RoPE + v transpose, per kv group ----
                q_sb = qp.tile([128, HPC, CW], bf16, tag="q")
                for g in range(GPC):
                    stage = []
                    for sub in range(6):     # 4 q tiles, 1 k tile, 1 v tile
                        jt = 6 * g + sub
                        if jt < 2:
                            wq_sb = wq_pre[jt]
                        else:
                            wq_sb = wqp.tile([128, NET, 128], bf16, tag="wq")
                            nc.sync.dma_start(out=wq_sb, in_=wq_d[jt])
                        ps = pmm.tile([128, CW], f32, tag="mm")
                        for et in range(NET):
                            nc.tensor.matmul(
                                ps,
                                lhsT=wq_sb[:, et, :],
                                rhs=xt_h[et // (NET // 2)][:, et % (NET // 2), :],
                                start=(et == 0),
                                stop=(et == NET - 1),
                            )
                        st = stp.tile([128, CW], f32, tag="st")
                        nc.scalar.copy(st, ps)
                        stage.append(st)
                    for sub in range(QPK + 1):  # RoPE on 4 q tiles + 1 k tile
                        stq = stage[sub]
                        is_q = sub < QPK
                        # interleaved pair-swap via partition-strided DMA
                        sw = rtp.tile([128, CW], f32, tag="sw")
                        nc.gpsimd.dma_start(out=sw[0::2, :], in_=stq[1::2, :])
                        nc.gpsimd.dma_start(out=sw[1::2, :], in_=stq[0::2, :])
                        tmp = rtp.tile([128, CW], f32, tag="rt")
                        nc.vector.tensor_mul(tmp, sw, sq_sb if is_q else sk_sb)
                        nc.vector.tensor_mul(stq, stq, cq_sb if is_q else ck_sb)
                        if is_q:
                            nc.vector.tensor_add(q_sb[:, QPK * g + sub, :], stq, tmp)
                        else:
                            nc.vector.tensor_add(k_sb[g][:, csl], stq, tmp)
                    stv = stage[5]
                    for u in range(CW // 128):
                        tp_ = pmm.tile([128, CW], f32, tag="mm")
                        nc.tensor.transpose(
                            tp_[:, :128], stv[:, u * 128:(u + 1) * 128], ident
                        )
                        nc.scalar.copy(
                            v_sb[g][:, (CW // 128) * c + u, :], tp_[:, :128]
                        )

                # ---- attention for q strip c (flash-style, no max) ----
                njt2 = (CW // 128) * (c + 1)     # causal: k tiles 0..4c+3
                ctx_sb = ctxp.tile([128, HPC, CW], bf16, tag="ctx")
                for g in range(GPC):
                    for hq in range(QPK):
                        h = QPK * g + hq
                        if c > 0:
                            # software pipeline: strip c-1's output projection
                            # block (ec = h) fills PE while ACT/DVE run softmax
                            emit_wo_block(c - 1, h, prev_ctx)
                        ctx_ps = pacc.tile([128, CW], f32, tag="acc")
                        accL = accp.tile([128, CW], bf16, tag="accL")
                        accR = accp.tile([128, CW], bf16, tag="accR")
                        for jp in range(njt2 // 2):
                            qk = pqk.tile([128, 2 * CW], f32, tag="qk")
                            offs = []
                            for half in range(2):
                                j2 = 2 * jp + half
                                diag = j2 >= njt2 - (CW // 128)
                                o = 128 * (j2 - (njt2 - (CW // 128))) if diag else 0
                                offs.append((j2, o, diag))
                                nc.tensor.matmul(
                                    qk[:, half * CW + o:(half + 1) * CW],
                                    lhsT=k_sb[g][:, j2 * 128:(j2 + 1) * 128],
                                    rhs=q_sb[:, h, o:],
                                    start=True, stop=True,
                                )
                            # one exp over the whole 2-bank pair; the masked
                            # (garbage) columns of diagonal tiles are simply
                            # never read downstream
                            at = atp.tile([128, 2 * CW], bf16, tag="at")
                            nc.scalar.activation(
                                at, qk, mybir.ActivationFunctionType.Exp
                            )
                            for half in range(2):
                                j2, o, diag = offs[half]
                                base = half * CW
                                if diag:
                                    nc.vector.tensor_mul(
                                        at[:, base + o:base + o + 128],
                                        at[:, base + o:base + o + 128],
                                        mk_sb[:, 0, :128],
                                    )
                                nc.tensor.matmul(
                                    ctx_ps[:, o:],
                                    lhsT=v_sb[g][:, j2, :],
                                    rhs=at[:, base + o:base + CW],
                                    start=(j2 == 0), stop=(j2 == njt2 - 1),
                                )
                                # fold into the row-sum accumulators: L-halves
                                # on DVE, R-halves on GpSimd (parallel chains)
                                eng = nc.vector if half == 0 else nc.gpsimd
                                acc_t = accL if half == 0 else accR
                                if jp == 0:
                                    if o:
                                        nc.gpsimd.memset(acc_t[:, :o], 0.0)
                                    eng.tensor_copy(
                                        acc_t[:, o:], at[:, base + o:base + CW]
                                    )
                                else:
                                    eng.tensor_add(
                                        acc_t[:, o:], acc_t[:, o:],
                                        at[:, base + o:base + CW],
                                    )
                        # single ones-reduction per head: sums = colsum(accL+accR)
                        sums = pqk.tile([128, 2 * CW], f32, tag="qk", name="sums")
                        nc.tensor.matmul(sums[:, :CW], lhsT=ones_sb, rhs=accL,
                                         start=True, stop=False)
                        nc.tensor.matmul(sums[:, :CW], lhsT=ones_sb, rhs=accR,
                                         start=False, stop=True)
                        rc = rcp.tile([128, CW], f32, tag="rc")
                        nc.vector.reciprocal_approx_fast(out=rc, in_=sums[:, :CW])
                        nc.vector.tensor_mul(ctx_sb[:, h, :], ctx_ps, rc)

                prev_ctx = ctx_sb

            # drain: output projection for the final strip
            for ec in range(ECN):
                emit_wo_block(SC - 1, ec, prev_ctx, drain=True)
    nc.finalize()
    return nc


def _rope_tables(scale):
    inv = 1.0 / (ROPE_BASE ** (np.arange(0, D, 2, dtype=np.float64) / D))
    ang = np.arange(S, dtype=np.float64)[None, :] * inv[:, None]    # [D/2, S]
    C = np.empty((D, S), np.float32)
    Sx = np.empty((D, S), np.float32)
    C[0::2] = np.cos(ang)
    C[1::2] = np.cos(ang)
    Sx[0::2] = -np.sin(ang)
    Sx[1::2] = np.sin(ang)
    return (C * scale).astype(np.float32), (Sx * scale).astype(np.float32)


def _host_inputs(x, wqkv, wo):
    """Shard + retile inputs for the 8 cores. Core c = 4*db + t."""
    cq, sq = _rope_tables(D ** -0.5)
    ck, sk = _rope_tables(1.0)

    # causal mask tiles in scores^T layout: keep when jj + 128*r <= ii
    jj = np.arange(128)[:, None]
    ii = np.arange(CW)[None, :]
    mk = np.empty((128, SC, CW), np_bf16)
    for r in range(SC):
        mk[:, r, :] = (jj + 128 * r <= ii).astype(np_bf16)

    xts = []
    for db in range(DP):
        xT = np.ascontiguousarray(x[db].T)                 # [E, S]
        t = xT.reshape(NET, 128, SC, CW).transpose(2, 1, 0, 3)
        xts.append(np.ascontiguousarray(t.astype(np_bf16)))

    wqs, wos = [], []
    rows = (E + 2 * KVH * D) // TP
    for t in range(TP):
        wT = np.ascontiguousarray(wqkv[rows * t:rows * (t + 1)].T)   # [E, 1536]
        wq_t = wT.reshape(NET, 128, NJT, 128).transpose(2, 1, 0, 3)
        wqs.append(np.ascontiguousarray(wq_t.astype(np_bf16)))
        woT = np.ascontiguousarray(wo[:, 1024 * t:1024 * (t + 1)].T)  # [1024, E]
        wo_t = woT.reshape(FT, 128, ECN, CW).transpose(2, 1, 0, 3)
        wos.append(np.ascontiguousarray(wo_t.astype(np_bf16)))

    in_maps = []
    for c in range(NCORES):
        db, t = divmod(c, TP)
        in_maps.append({
            "xt": xts[db], "wq": wqs[t], "wo": wos[t],
            "cq": cq, "sq": sq, "ck": ck, "sk": sk,
            "mk": mk,
        })
    return in_maps


def kernel(x, wqkv, wo):
    x = np.asarray(x, np.float32)
    wqkv = np.asarray(wqkv, np.float32)
    wo = np.asarray(wo, np.float32)

    if "nc" not in _built:
        _built["nc"] = _build_nc()
    nc = _built["nc"]

    in_maps = _host_inputs(x, wqkv, wo)
    res = run_bass_kernel_spmd(nc, in_maps, core_ids=list(range(NCORES)))
    globals()["_last_results"] = res

    out = np.zeros((B, S, E), np.float32)
    for c in range(NCORES):
        db = c // TP
        o = np.asarray(res.results[c]["out"], dtype=np.float32)
        # [SC, ECN, 128, 4, CW] -> [SC, 4, 128, ECN, CW] -> [S, E]
        out[db] += o.transpose(0, 3, 2, 1, 4).reshape(S, E)
    return out


# revision 9
# speedup vs baseline: 1.1779x; 1.1779x over previous
"""Bass/Trainium2 kernel for GQA transformer block (nn_GQA_84353157694016).

v2: fully software-pipelined schedule.  Each "attention phase" for strip c
interleaves, per head: the previous strip's output-projection block, the
NEXT strip's qkv-projection j-tiles (weights prefetched one head ahead),
and the head's own score/softmax/AV work.  This removes the serial
qkv-phase <-> attention-phase transitions entirely: the PE sees a uniform
mix of dense matmul work while ACT/DVE/GpSimd run softmax underneath.

Other deltas vs v1:
  - v is transposed by the DMA xbar (dma_start_transpose) straight from the
    projection PSUM copy, freeing the PE transposes and 3 scalar copies.
  - all input loads ride sync/scalar/gpsimd queues so no engine FIFO ever
    blocks behind a transfer it doesn't need.
"""

import os
import sys

import numpy as np
import ml_dtypes

for _p in ("/opt/trn_rl_repo",):
    if _p not in sys.path and os.path.isdir(_p):
        sys.path.append(_p)

import concourse.bass as bass
import concourse.tile as tile
from concourse import bacc, mybir
from concourse.bass_utils import run_bass_kernel_spmd
from concourse.masks import make_identity


def _install_ntff_hook():
    try:
        import antenv.axon_hooks  # noqa: F401
        return
    except ImportError:
        pass
    try:
        import types
        import antenv
        mod = types.ModuleType("antenv.axon_hooks")
        mod._hook = None
        mod.set_axon_ntff_profile_hook = lambda h: setattr(mod, "_hook", h)
        mod.get_axon_ntff_profile_hook = lambda: mod._hook
        sys.modules["antenv.axon_hooks"] = mod
        antenv.axon_hooks = mod
        from trn_agent_boot.trn_boot import _ntff_profile_via_ctypes
        so = "/opt/axon/libaxon_pjrt.so"
        if os.path.exists(so):
            mod._hook = _ntff_profile_via_ctypes(so)
    except Exception:
        pass


_install_ntff_hook()

# problem constants
B, S, E = 2, 2048, 4096
H, KVH, D = 32, 8, 128
QPK = H // KVH                    # 4 q heads per kv group
ROPE_BASE = 10000.0

NCORES = 8
TP = 4                            # tensor-parallel width (heads)
DP = 2                            # data-parallel width (batch)

SC = 4                            # number of s-chunks == q strips
CW = S // SC                      # 512 chunk width
NJT = (E + 2 * KVH * D) // TP // 128   # 12 qkv row-tiles per core
NET = E // 128                    # 32 contraction tiles for qkv proj
GPC = KVH // TP                   # 2 kv groups per core
HPC = H // TP                     # 8 q heads per core
FT = HPC * D // 128               # 8 local ctx feature tiles
ECN = E // CW                     # 8 output e-chunks

# qkv j-tiles of strip c+1 whose matmuls run inside head h of strip c's
# attention; the DMA for a tile is issued one head earlier.
STEPS = (2, 2, 1, 1, 2, 2, 1, 1)

f32 = mybir.dt.float32
bf16 = mybir.dt.bfloat16
np_bf16 = ml_dtypes.bfloat16

_built = {}


def _build_nc():
    nc = bacc.Bacc("TRN2", target_bir_lowering=False)

    xt_d = nc.dram_tensor("xt", [SC, 128, NET, CW], bf16, kind="ExternalInput")
    wq_d = nc.dram_tensor("wq", [NJT, 128, NET, 128], bf16, kind="ExternalInput")
    wo_d = nc.dram_tensor("wo", [ECN, 128, FT, CW], bf16, kind="ExternalInput")
    cq_d = nc.dram_tensor("cq", [128, S], f32, kind="ExternalInput")
    sq_d = nc.dram_tensor("sq", [128, S], f32, kind="ExternalInput")
    ck_d = nc.dram_tensor("ck", [128, S], f32, kind="ExternalInput")
    sk_d = nc.dram_tensor("sk", [128, S], f32, kind="ExternalInput")
    mk_d = nc.dram_tensor("mk", [128, SC, CW], bf16, kind="ExternalInput")
    out_d = nc.dram_tensor("out", [SC, ECN, 128, CW // 128, CW], bf16,
                           kind="ExternalOutput")

    from contextlib import ExitStack
    with tile.TileContext(nc) as tc:
        with ExitStack() as _stk:
            def _pool(*a, **kw):
                return _stk.enter_context(tc.tile_pool(*a, **kw))
            constp = _pool(name="const", bufs=1)
            tabp = _pool(name="tab", bufs=2)
            xtp = _pool(name="xt", bufs=3)
            wqp = _pool(name="wq", bufs=2)
            stp = _pool(name="st", bufs=7)
            rtp = _pool(name="rt", bufs=2)
            qp = _pool(name="q", bufs=2)
            kvp = _pool(name="kv", bufs=1)
            atp = _pool(name="at", bufs=4)
            accp = _pool(name="acc", bufs=2)
            ctxp = _pool(name="ctx", bufs=2)
            wop = _pool(name="wop", bufs=4)
            obp = _pool(name="ob", bufs=1)
            vtp = _pool(name="vt", bufs=1)
            rcp = _pool(name="rc", bufs=1)
            pmm = _pool(name="pmm", bufs=2, space="PSUM")
            pqk = _pool(name="pqk", bufs=2, space="PSUM")
            pacc = _pool(name="pacc", bufs=2, space="PSUM")

            def issue_wo(ec, drain=False):
                """Prefetch the wo weight block for e-chunk ec."""
                wo_sb = wop.tile([128, FT, CW], bf16, tag="wo", name="wo_sb")
                if drain:
                    _e = (nc.sync, nc.scalar, nc.gpsimd)
                    engs = (_e[(2 * ec) % 3], _e[(2 * ec + 1) % 3])
                else:
                    engs = (nc.scalar, nc.gpsimd)
                engs[0].dma_start(out=wo_sb[:, :FT // 2, :],
                                  in_=wo_d[ec, :, :FT // 2, :])
                engs[1].dma_start(out=wo_sb[:, FT // 2:, :],
                                  in_=wo_d[ec, :, FT // 2:, :])
                return wo_sb

            def emit_wo_block(cs, ec, ctx_tiles, wo_sb):
                """Output-projection block: out[strip cs, ec] += ctx @ woT."""
                ob = obp.tile([128, CW // 128, CW], bf16, tag="ob", name="ob")
                for sti in range(CW // 128):
                    ps = pacc.tile([128, CW], f32, tag="acc", name="wo_ps")
                    for ft in range(FT):
                        nc.tensor.matmul(
                            ps,
                            lhsT=ctx_tiles[:, ft, sti * 128:(sti + 1) * 128],
                            rhs=wo_sb[:, ft, :],
                            start=(ft == 0),
                            stop=(ft == FT - 1),
                        )
                    nc.vector.tensor_copy(ob[:, sti, :], ps)
                nc.gpsimd.dma_start(out=out_d[cs, ec], in_=ob)

            # constants
            ones_sb = constp.tile([128, 128], bf16, tag="ones")
            nc.vector.memset(ones_sb, 1.0)
            mk_sb = constp.tile([128, SC, CW], bf16, tag="mk")

            # persistent k (transposed) and v (natural) per kv group, bf16
            k_sb = [kvp.tile([128, S], bf16, tag=f"k{g}", name=f"k{g}")
                    for g in range(GPC)]
            v_sb = [kvp.tile([128, S // 128, 128], bf16, tag=f"v{g}", name=f"v{g}")
                    for g in range(GPC)]

            def start_chunk(c):
                """Allocate per-strip tiles + issue the first input DMAs."""
                st = {"c": c, "stage": {0: [], 1: []}, "wq": {}}
                for pre in range(2):
                    w_ = wqp.tile([128, NET, 128], bf16, tag="wq", name="wq_pre")
                    nc.sync.dma_start(out=w_, in_=wq_d[pre])
                    st["wq"][pre] = w_
                xh = []
                for half, eng in ((0, nc.scalar), (1, nc.gpsimd)):
                    xh_t = xtp.tile([128, NET // 2, CW], bf16, tag="xt", name="xh")
                    eng.dma_start(
                        out=xh_t,
                        in_=xt_d[c, :, half * (NET // 2):(half + 1) * (NET // 2), :],
                    )
                    xh.append(xh_t)
                st["xh"] = xh
                csl = slice(c * CW, (c + 1) * CW)
                tabs = []
                for src in (cq_d, sq_d, ck_d, sk_d):
                    t_ = tabp.tile([128, CW], f32, tag=f"tab{len(tabs)}",
                                   name="tab")
                    nc.gpsimd.dma_start(out=t_, in_=src[:, csl])
                    tabs.append(t_)
                st["tabs"] = tabs
                st["q_sb"] = qp.tile([128, HPC, CW], bf16, tag="q", name="q_sb")
                return st

            def issue_wq(st, jt):
                """Prefetch the weight tile for j-tile jt (one head ahead)."""
                if jt >= 2 and jt < NJT:
                    w_ = wqp.tile([128, NET, 128], bf16, tag="wq", name="wq_jt")
                    nc.sync.dma_start(out=w_, in_=wq_d[jt])
                    st["wq"][jt] = w_

            def emit_qkv_step(st, jt):
                """Matmuls + drain for one qkv j-tile of strip st['c']."""
                c = st["c"]
                g, sub = divmod(jt, 6)
                wq_sb = st["wq"].pop(jt)
                ps = pmm.tile([128, CW], f32, tag="mm", name="qkv_ps")
                for et in range(NET):
                    nc.tensor.matmul(
                        ps,
                        lhsT=wq_sb[:, et, :],
                        rhs=st["xh"][et // (NET // 2)][:, et % (NET // 2), :],
                        start=(et == 0),
                        stop=(et == NET - 1),
                    )
                if sub == 5:
                    # v tile: cast to bf16 once, then DMA-xbar transpose into
                    # the persistent [kpos, d] v buffer (no PE transpose)
                    vt = vtp.tile([128, CW], bf16, tag="vt", name="vt")
                    nc.scalar.copy(vt, ps)
                    for u in range(CW // 128):
                        nc.sync.dma_start_transpose(
                            out=v_sb[g][:, (CW // 128) * c + u, :],
                            in_=vt[:, u * 128:(u + 1) * 128],
                        )
                else:
                    s_t = stp.tile([128, CW], f32, tag="st", name="st")
                    nc.scalar.copy(s_t, ps)
                    st["stage"][g].append(s_t)

            def emit_rope(st, g):
                """RoPE for the 4 q tiles + 1 k tile of group g."""
                c = st["c"]
                csl = slice(c * CW, (c + 1) * CW)
                cq_sb, sq_sb, ck_sb, sk_sb = st["tabs"]
                stage = st["stage"][g]
                for sub in range(QPK + 1):
                    stq = stage[sub]
                    is_q = sub < QPK
                    sw = rtp.tile([128, CW], f32, tag="sw")
                    nc.gpsimd.dma_start(out=sw[0::2, :], in_=stq[1::2, :])
                    nc.gpsimd.dma_start(out=sw[1::2, :], in_=stq[0::2, :])
                    tmp = rtp.tile([128, CW], f32, tag="rt")
                    nc.vector.tensor_mul(tmp, sw, sq_sb if is_q else sk_sb)
                    nc.vector.tensor_mul(stq, stq, cq_sb if is_q else ck_sb)
                    if is_q:
                        nc.vector.tensor_add(
                            st["q_sb"][:, QPK * g + sub, :], stq, tmp)
                    else:
                        nc.vector.tensor_add(k_sb[g][:, csl], stq, tmp)

            def emit_attention_head(c, g, h, q_sb, ctx_sb):
                """Scores + softmax + AV for one head of strip c."""
                njt2 = (CW // 128) * (c + 1)
                ctx_ps = pacc.tile([128, CW], f32, tag="acc", name="ctx_ps")
                accL = accp.tile([128, CW], bf16, tag="accL")
                accR = accp.tile([128, CW], bf16, tag="accR")
                for jp in range(njt2 // 2):
                    qk = pqk.tile([128, 2 * CW], f32, tag="qk", name="qk")
                    offs = []
                    for half in range(2):
                        j2 = 2 * jp + half
                        diag = j2 >= njt2 - (CW // 128)
                        o = 128 * (j2 - (njt2 - (CW // 128))) if diag else 0
                        offs.append((j2, o, diag))
                        nc.tensor.matmul(
                            qk[:, half * CW + o:(half + 1) * CW],
                            lhsT=k_sb[g][:, j2 * 128:(j2 + 1) * 128],
                            rhs=q_sb[:, h, o:],
                            start=True, stop=True,
                        )
                    # one exp over the whole 2-bank pair; masked (garbage)
                    # columns of diagonal tiles are never read downstream
                    at = atp.tile([128, 2 * CW], bf16, tag="at")
                    nc.scalar.activation(
                        at, qk, mybir.ActivationFunctionType.Exp
                    )
                    for half in range(2):
                        j2, o, diag = offs[half]
                        base = half * CW
                        if diag:
                            nc.vector.tensor_mul(
                                at[:, base + o:base + o + 128],
                                at[:, base + o:base + o + 128],
                                mk_sb[:, 0, :128],
                            )
                        nc.tensor.matmul(
                            ctx_ps[:, o:],
                            lhsT=v_sb[g][:, j2, :],
                            rhs=at[:, base + o:base + CW],
                            start=(j2 == 0), stop=(j2 == njt2 - 1),
                        )
                        # row-sum accumulators: two independent DVE chains
                        eng = nc.vector
                        acc_t = accL if half == 0 else accR
                        if jp == 0:
                            if o:
                                nc.gpsimd.memset(acc_t[:, :o], 0.0)
                            eng.tensor_copy(
                                acc_t[:, o:], at[:, base + o:base + CW]
                            )
                        else:
                            eng.tensor_add(
                                acc_t[:, o:], acc_t[:, o:],
                                at[:, base + o:base + CW],
                            )
                sums = pqk.tile([128, 2 * CW], f32, tag="qk", name="sums")
                nc.tensor.matmul(sums[:, :CW], lhsT=ones_sb, rhs=accL,
                                 start=True, stop=False)
                nc.tensor.matmul(sums[:, :CW], lhsT=ones_sb, rhs=accR,
                                 start=False, stop=True)
                rc = rcp.tile([128, CW], f32, tag="rc")
                nc.vector.reciprocal_approx_fast(out=rc, in_=sums[:, :CW])
                nc.vector.tensor_mul(ctx_sb[:, h, :], ctx_ps, rc)

            # ---------------- main pipelined schedule ----------------
            cur = start_chunk(0)
            nc.sync.dma_start(out=mk_sb, in_=mk_d[:])
            for jt in range(NJT):
                issue_wq(cur, jt + 2 if jt + 2 < NJT else NJT)
                emit_qkv_step(cur, jt)
                if jt == 5:
                    emit_rope(cur, 0)
            emit_rope(cur, 1)

            # wo weight blocks are prefetched two head-slots ahead across the
            # whole flat sequence (3 interleaved phases + the drain)
            from collections import deque
            wo_tiles = deque()
            wo_ctr = [0]

            def wo_issue():
                i = wo_ctr[0]
                if i >= 4 * ECN:
                    return
                wo_ctr[0] += 1
                wo_tiles.append(issue_wo(i % ECN, drain=(i >= 3 * ECN)))

            prev_ctx = None
            for c in range(SC):
                nxt = start_chunk(c + 1) if c < SC - 1 else None
                ctx_sb = ctxp.tile([128, HPC, CW], bf16, tag="ctx")
                jt_mm = 0      # next j-tile (of strip c+1) to run matmuls for
                jt_dma = 2     # next j-tile to prefetch weights for
                for g in range(GPC):
                    for hq in range(QPK):
                        h = QPK * g + hq
                        if c > 0:
                            emit_wo_block(c - 1, h, prev_ctx,
                                          wo_tiles.popleft())
                        emit_attention_head(c, g, h, cur["q_sb"], ctx_sb)
                        if nxt is not None:
                            for _ in range(STEPS[h]):
                                issue_wq(nxt, jt_dma)
                                jt_dma += 1
                                emit_qkv_step(nxt, jt_mm)
                                jt_mm += 1
                            if h == 3:
                                emit_rope(nxt, 0)
                            if h == QPK * GPC - 1:
                                emit_rope(nxt, 1)
                        if c > 0 or h >= QPK * GPC - 2:
                            wo_issue()
                prev_ctx = ctx_sb
                if nxt is not None:
                    cur = nxt

            # drain: output projection for the final strip
            for ec in range(ECN):
                emit_wo_block(SC - 1, ec, prev_ctx, wo_tiles.popleft())
                wo_issue()
    nc.finalize()
    return nc


def _rope_tables(scale):
    inv = 1.0 / (ROPE_BASE ** (np.arange(0, D, 2, dtype=np.float64) / D))
    ang = np.arange(S, dtype=np.float64)[None, :] * inv[:, None]    # [D/2, S]
    C = np.empty((D, S), np.float32)
    Sx = np.empty((D, S), np.float32)
    C[0::2] = np.cos(ang)
    C[1::2] = np.cos(ang)
    Sx[0::2] = -np.sin(ang)
    Sx[1::2] = np.sin(ang)
    return (C * scale).astype(np.float32), (Sx * scale).astype(np.float32)


def _host_inputs(x, wqkv, wo):
    """Shard + retile inputs for the 8 cores. Core c = 4*db + t."""
    cq, sq = _rope_tables(D ** -0.5)
    ck, sk = _rope_tables(1.0)

    # causal mask tiles in scores^T layout: keep when jj + 128*r <= ii
    jj = np.arange(128)[:, None]
    ii = np.arange(CW)[None, :]
    mk = np.empty((128, SC, CW), np_bf16)
    for r in range(SC):
        mk[:, r, :] = (jj + 128 * r <= ii).astype(np_bf16)

    xts = []
    for db in range(DP):
        xT = np.ascontiguousarray(x[db].T)                 # [E, S]
        t = xT.reshape(NET, 128, SC, CW).transpose(2, 1, 0, 3)
        xts.append(np.ascontiguousarray(t.astype(np_bf16)))

    wqs, wos = [], []
    rows = (E + 2 * KVH * D) // TP
    for t in range(TP):
        wT = np.ascontiguousarray(wqkv[rows * t:rows * (t + 1)].T)   # [E, 1536]
        wq_t = wT.reshape(NET, 128, NJT, 128).transpose(2, 1, 0, 3)
        wqs.append(np.ascontiguousarray(wq_t.astype(np_bf16)))
        woT = np.ascontiguousarray(wo[:, 1024 * t:1024 * (t + 1)].T)  # [1024, E]
        wo_t = woT.reshape(FT, 128, ECN, CW).transpose(2, 1, 0, 3)
        wos.append(np.ascontiguousarray(wo_t.astype(np_bf16)))

    in_maps = []
    for c in range(NCORES):
        db, t = divmod(c, TP)
        in_maps.append({
            "xt": xts[db], "wq": wqs[t], "wo": wos[t],
            "cq": cq, "sq": sq, "ck": ck, "sk": sk,
            "mk": mk,
        })
    return in_maps


def kernel(x, wqkv, wo):
    x = np.asarray(x, np.float32)
    wqkv = np.asarray(wqkv, np.float32)
    wo = np.asarray(wo, np.float32)

    if "nc" not in _built:
        _built["nc"] = _build_nc()
    nc = _built["nc"]

    in_maps = _host_inputs(x, wqkv, wo)
    res = run_bass_kernel_spmd(nc, in_maps, core_ids=list(range(NCORES)))
    globals()["_last_results"] = res

    out = np.zeros((B, S, E), np.float32)
    for c in range(NCORES):
        db = c // TP
        o = np.asarray(res.results[c]["out"], dtype=np.float32)
        # [SC, ECN, 128, 4, CW] -> [SC, 4, 128, ECN, CW] -> [S, E]
        out[db] += o.transpose(0, 3, 2, 1, 4).reshape(S, E)
    return out


# revision 10
# speedup vs baseline: 1.2487x; 1.0601x over previous
"""Bass/Trainium2 kernel for GQA transformer block (nn_GQA_84353157694016).

v2: fully software-pipelined schedule.  Each "attention phase" for strip c
interleaves, per head: the previous strip's output-projection block, the
NEXT strip's qkv-projection j-tiles (weights prefetched one head ahead),
and the head's own score/softmax/AV work.  This removes the serial
qkv-phase <-> attention-phase transitions entirely: the PE sees a uniform
mix of dense matmul work while ACT/DVE/GpSimd run softmax underneath.

Other deltas vs v1:
  - v is transposed by the DMA xbar (dma_start_transpose) straight from the
    projection PSUM copy, freeing the PE transposes and 3 scalar copies.
  - all input loads ride sync/scalar/gpsimd queues so no engine FIFO ever
    blocks behind a transfer it doesn't need.
"""

import os
import sys

import numpy as np
import ml_dtypes

for _p in ("/opt/trn_rl_repo",):
    if _p not in sys.path and os.path.isdir(_p):
        sys.path.append(_p)

import concourse.bass as bass
import concourse.tile as tile
from concourse import bacc, mybir
from concourse.bass_utils import run_bass_kernel_spmd
from concourse.masks import make_identity


def _install_ntff_hook():
    try:
        import antenv.axon_hooks  # noqa: F401
        return
    except ImportError:
        pass
    try:
        import types
        import antenv
        mod = types.ModuleType("antenv.axon_hooks")
        mod._hook = None
        mod.set_axon_ntff_profile_hook = lambda h: setattr(mod, "_hook", h)
        mod.get_axon_ntff_profile_hook = lambda: mod._hook
        sys.modules["antenv.axon_hooks"] = mod
        antenv.axon_hooks = mod
        from trn_agent_boot.trn_boot import _ntff_profile_via_ctypes
        so = "/opt/axon/libaxon_pjrt.so"
        if os.path.exists(so):
            mod._hook = _ntff_profile_via_ctypes(so)
    except Exception:
        pass


_install_ntff_hook()

# problem constants
B, S, E = 2, 2048, 4096
H, KVH, D = 32, 8, 128
QPK = H // KVH                    # 4 q heads per kv group
ROPE_BASE = 10000.0

NCORES = 8
TP = 4                            # tensor-parallel width (heads)
DP = 2                            # data-parallel width (batch)

SC = 4                            # number of s-chunks == q strips
CW = S // SC                      # 512 chunk width
NJT = (E + 2 * KVH * D) // TP // 128   # 12 qkv row-tiles per core
NET = E // 128                    # 32 contraction tiles for qkv proj
GPC = KVH // TP                   # 2 kv groups per core
HPC = H // TP                     # 8 q heads per core
FT = HPC * D // 128               # 8 local ctx feature tiles
ECN = E // CW                     # 8 output e-chunks

# qkv j-tiles of strip c+1 whose matmuls run inside head h of strip c's
# attention; the DMA for a tile is issued one head earlier.
STEPS = (2, 1, 2, 1, 2, 1, 2, 1)

f32 = mybir.dt.float32
bf16 = mybir.dt.bfloat16
np_bf16 = ml_dtypes.bfloat16

_built = {}


def _build_nc():
    nc = bacc.Bacc("TRN2", target_bir_lowering=False)

    xt_d = nc.dram_tensor("xt", [SC, 128, NET, CW], bf16, kind="ExternalInput")
    wq_d = nc.dram_tensor("wq", [NJT, 128, NET, 128], bf16, kind="ExternalInput")
    wo_d = nc.dram_tensor("wo", [ECN, 128, FT, CW], bf16, kind="ExternalInput")
    cq_d = nc.dram_tensor("cq", [128, S], f32, kind="ExternalInput")
    sq_d = nc.dram_tensor("sq", [128, S], f32, kind="ExternalInput")
    ck_d = nc.dram_tensor("ck", [128, S], f32, kind="ExternalInput")
    sk_d = nc.dram_tensor("sk", [128, S], f32, kind="ExternalInput")
    mk_d = nc.dram_tensor("mk", [128, SC, CW], bf16, kind="ExternalInput")
    out_d = nc.dram_tensor("out", [SC, ECN, 128, CW // 128, CW], bf16,
                           kind="ExternalOutput")

    from contextlib import ExitStack
    with tile.TileContext(nc) as tc:
        with ExitStack() as _stk:
            def _pool(*a, **kw):
                return _stk.enter_context(tc.tile_pool(*a, **kw))
            constp = _pool(name="const", bufs=1)
            tabp = _pool(name="tab", bufs=2)
            xtp = _pool(name="xt", bufs=5)
            wqp = _pool(name="wq", bufs=3)
            stp = _pool(name="st", bufs=6)
            rtp = _pool(name="rt", bufs=2)
            qp = _pool(name="q", bufs=2)
            kvp = _pool(name="kv", bufs=1)
            atp = _pool(name="at", bufs=3)
            accp = _pool(name="acc", bufs=2)
            ctxp = _pool(name="ctx", bufs=2)
            wop = _pool(name="wop", bufs=4)
            obp = _pool(name="ob", bufs=2)
            vtp = _pool(name="vt", bufs=1)
            rcp = _pool(name="rc", bufs=1)
            pmm = _pool(name="pmm", bufs=2, space="PSUM")
            pqk = _pool(name="pqk", bufs=2, space="PSUM")
            pacc = _pool(name="pacc", bufs=2, space="PSUM")

            def issue_wo(ec, drain=False):
                """Prefetch the wo weight block for e-chunk ec."""
                wo_sb = wop.tile([128, FT, CW], bf16, tag="wo", name="wo_sb")
                if drain:
                    _e = (nc.sync, nc.scalar, nc.gpsimd)
                    engs = (_e[(2 * ec) % 3], _e[(2 * ec + 1) % 3])
                else:
                    engs = (nc.scalar, nc.gpsimd)
                engs[0].dma_start(out=wo_sb[:, :FT // 2, :],
                                  in_=wo_d[ec, :, :FT // 2, :])
                engs[1].dma_start(out=wo_sb[:, FT // 2:, :],
                                  in_=wo_d[ec, :, FT // 2:, :])
                return wo_sb

            def emit_wo_block(cs, ec, ctx_tiles, wo_sb):
                """Output-projection block: out[strip cs, ec] += ctx @ woT."""
                ob = obp.tile([128, CW // 128, CW], bf16, tag="ob", name="ob")
                for sti in range(CW // 128):
                    ps = pacc.tile([128, CW], f32, tag="acc", name="wo_ps")
                    for ft in range(FT):
                        nc.tensor.matmul(
                            ps,
                            lhsT=ctx_tiles[:, ft, sti * 128:(sti + 1) * 128],
                            rhs=wo_sb[:, ft, :],
                            start=(ft == 0),
                            stop=(ft == FT - 1),
                        )
                    nc.vector.tensor_copy(ob[:, sti, :], ps)
                nc.gpsimd.dma_start(out=out_d[cs, ec], in_=ob)

            # constants
            ones_sb = constp.tile([128, 128], bf16, tag="ones")
            nc.vector.memset(ones_sb, 1.0)
            mk_sb = constp.tile([128, SC, CW], bf16, tag="mk")

            # persistent k (transposed) and v (natural) per kv group, bf16
            k_sb = [kvp.tile([128, S], bf16, tag=f"k{g}", name=f"k{g}")
                    for g in range(GPC)]
            v_sb = [kvp.tile([128, S // 128, 128], bf16, tag=f"v{g}", name=f"v{g}")
                    for g in range(GPC)]

            def start_chunk(c):
                """Allocate per-strip tiles + issue the first input DMAs."""
                st = {"c": c, "stage": {0: [], 1: []}, "wq": {}}
                for pre in range(2):
                    w_ = wqp.tile([128, NET, 128], bf16, tag="wq", name="wq_pre")
                    nc.sync.dma_start(out=w_[:, :NET // 2, :],
                                      in_=wq_d[pre, :, :NET // 2, :])
                    (nc.scalar, nc.gpsimd)[pre].dma_start(
                        out=w_[:, NET // 2:, :],
                        in_=wq_d[pre, :, NET // 2:, :])
                    st["wq"][pre] = w_
                xh = []
                NQ = NET // 4
                for qt in range(4):
                    xh_t = xtp.tile([128, NQ, CW], bf16, tag="xt", name="xh")
                    (nc.scalar, nc.gpsimd)[qt % 2].dma_start(
                        out=xh_t,
                        in_=xt_d[c, :, qt * NQ:(qt + 1) * NQ, :],
                    )
                    xh.append(xh_t)
                st["xh"] = xh
                csl = slice(c * CW, (c + 1) * CW)
                tabs = []
                for src in (cq_d, sq_d, ck_d, sk_d):
                    t_ = tabp.tile([128, CW], f32, tag=f"tab{len(tabs)}",
                                   name="tab")
                    nc.gpsimd.dma_start(out=t_, in_=src[:, csl])
                    tabs.append(t_)
                st["tabs"] = tabs
                st["q_sb"] = qp.tile([128, HPC, CW], bf16, tag="q", name="q_sb")
                return st

            def issue_wq(st, jt):
                """Prefetch the weight tile for j-tile jt (two heads ahead)."""
                if jt >= 2 and jt < NJT:
                    w_ = wqp.tile([128, NET, 128], bf16, tag="wq", name="wq_jt")
                    e2 = (nc.scalar, nc.gpsimd)[jt % 2]
                    nc.sync.dma_start(out=w_[:, :NET // 2, :],
                                      in_=wq_d[jt, :, :NET // 2, :])
                    e2.dma_start(out=w_[:, NET // 2:, :],
                                 in_=wq_d[jt, :, NET // 2:, :])
                    st["wq"][jt] = w_

            def emit_qkv_step(st, jt):
                """Matmuls + drain for one qkv j-tile of strip st['c']."""
                c = st["c"]
                g, sub = divmod(jt, 6)
                wq_sb = st["wq"].pop(jt)
                ps = pmm.tile([128, CW], f32, tag="mm", name="qkv_ps")
                for et in range(NET):
                    nc.tensor.matmul(
                        ps,
                        lhsT=wq_sb[:, et, :],
                        rhs=st["xh"][et // (NET // 4)][:, et % (NET // 4), :],
                        start=(et == 0),
                        stop=(et == NET - 1),
                    )
                if sub == 5:
                    # v tile: cast to bf16 once, then DMA-xbar transpose into
                    # the persistent [kpos, d] v buffer (no PE transpose)
                    vt = vtp.tile([128, CW], bf16, tag="vt", name="vt")
                    nc.scalar.copy(vt, ps)
                    for u in range(CW // 128):
                        nc.sync.dma_start_transpose(
                            out=v_sb[g][:, (CW // 128) * c + u, :],
                            in_=vt[:, u * 128:(u + 1) * 128],
                        )
                else:
                    s_t = stp.tile([128, CW], f32, tag="st", name="st")
                    nc.scalar.copy(s_t, ps)
                    st["stage"][g].append(s_t)

            def emit_rope(st, g):
                """RoPE for the 4 q tiles + 1 k tile of group g."""
                c = st["c"]
                csl = slice(c * CW, (c + 1) * CW)
                cq_sb, sq_sb, ck_sb, sk_sb = st["tabs"]
                stage = st["stage"][g]
                for sub in range(QPK + 1):
                    stq = stage[sub]
                    is_q = sub < QPK
                    sw = rtp.tile([128, CW], f32, tag="sw")
                    nc.gpsimd.dma_start(out=sw[0::2, :], in_=stq[1::2, :])
                    nc.gpsimd.dma_start(out=sw[1::2, :], in_=stq[0::2, :])
                    tmp = rtp.tile([128, CW], f32, tag="rt")
                    nc.vector.tensor_mul(tmp, sw, sq_sb if is_q else sk_sb)
                    nc.vector.tensor_mul(stq, stq, cq_sb if is_q else ck_sb)
                    if is_q:
                        nc.vector.tensor_add(
                            st["q_sb"][:, QPK * g + sub, :], stq, tmp)
                    else:
                        nc.vector.tensor_add(k_sb[g][:, csl], stq, tmp)

            def emit_attention_head(c, g, h, q_sb, ctx_sb):
                """Scores + softmax + AV for one head of strip c."""
                njt2 = (CW // 128) * (c + 1)
                ctx_ps = pacc.tile([128, CW], f32, tag="acc", name="ctx_ps")
                accL = accp.tile([128, CW], bf16, tag="accL")
                accR = accp.tile([128, CW], bf16, tag="accR")
                for jp in range(njt2 // 2):
                    qk = pqk.tile([128, 2 * CW], f32, tag="qk", name="qk")
                    offs = []
                    for half in range(2):
                        j2 = 2 * jp + half
                        diag = j2 >= njt2 - (CW // 128)
                        o = 128 * (j2 - (njt2 - (CW // 128))) if diag else 0
                        offs.append((j2, o, diag))
                        nc.tensor.matmul(
                            qk[:, half * CW + o:(half + 1) * CW],
                            lhsT=k_sb[g][:, j2 * 128:(j2 + 1) * 128],
                            rhs=q_sb[:, h, o:],
                            start=True, stop=True,
                        )
                    # one exp over the whole 2-bank pair; masked (garbage)
                    # columns of diagonal tiles are never read downstream
                    at = atp.tile([128, 2 * CW], bf16, tag="at")
                    nc.scalar.activation(
                        at, qk, mybir.ActivationFunctionType.Exp
                    )
                    for half in range(2):
                        j2, o, diag = offs[half]
                        base = half * CW
                        if diag:
                            nc.vector.tensor_mul(
                                at[:, base + o:base + o + 128],
                                at[:, base + o:base + o + 128],
                                mk_sb[:, 0, :128],
                            )
                        nc.tensor.matmul(
                            ctx_ps[:, o:],
                            lhsT=v_sb[g][:, j2, :],
                            rhs=at[:, base + o:base + CW],
                            start=(j2 == 0), stop=(j2 == njt2 - 1),
                        )
                        # row-sum accumulators: two independent DVE chains
                        eng = nc.vector
                        acc_t = accL if half == 0 else accR
                        if jp == 0:
                            if o:
                                nc.gpsimd.memset(acc_t[:, :o], 0.0)
                            eng.tensor_copy(
                                acc_t[:, o:], at[:, base + o:base + CW]
                            )
                        else:
                            eng.tensor_add(
                                acc_t[:, o:], acc_t[:, o:],
                                at[:, base + o:base + CW],
                            )
                sums = pqk.tile([128, 2 * CW], f32, tag="qk", name="sums")
                nc.tensor.matmul(sums[:, :CW], lhsT=ones_sb, rhs=accL,
                                 start=True, stop=False)
                nc.tensor.matmul(sums[:, :CW], lhsT=ones_sb, rhs=accR,
                                 start=False, stop=True)
                rc = rcp.tile([128, CW], f32, tag="rc")
                nc.vector.reciprocal_approx_fast(out=rc, in_=sums[:, :CW])
                nc.vector.tensor_mul(ctx_sb[:, h, :], ctx_ps, rc)

            # ---------------- main pipelined schedule ----------------
            cur = start_chunk(0)
            nc.sync.dma_start(out=mk_sb, in_=mk_d[:])
            for jt in range(NJT):
                issue_wq(cur, jt + 2 if jt + 2 < NJT else NJT)
                emit_qkv_step(cur, jt)
                if jt == 5:
                    emit_rope(cur, 0)
            emit_rope(cur, 1)

            # wo weight blocks are prefetched two head-slots ahead across the
            # whole flat sequence (3 interleaved phases + the drain)
            from collections import deque
            wo_tiles = deque()
            wo_ctr = [0]

            def wo_issue():
                i = wo_ctr[0]
                if i >= 4 * ECN:
                    return
                wo_ctr[0] += 1
                wo_tiles.append(issue_wo(i % ECN, drain=(i >= 3 * ECN)))

            prev_ctx = None
            for c in range(SC):
                nxt = start_chunk(c + 1) if c < SC - 1 else None
                ctx_sb = ctxp.tile([128, HPC, CW], bf16, tag="ctx")
                jt_mm = 0      # next j-tile (of strip c+1) to run matmuls for
                jt_dma = [2]   # next j-tile to prefetch weights for
                cum = [0]
                for shd in STEPS:
                    cum.append(cum[-1] + shd)
                for g in range(GPC):
                    for hq in range(QPK):
                        h = QPK * g + hq
                        if nxt is not None:
                            # prefetch weights ~one head ahead, capped at the
                            # pool depth so a DMA never camps on a full queue
                            tgt = min(NJT, cum[min(h + 2, len(STEPS))],
                                      jt_mm + 3)
                            while jt_dma[0] < tgt:
                                issue_wq(nxt, jt_dma[0])
                                jt_dma[0] += 1
                        if c > 0:
                            emit_wo_block(c - 1, h, prev_ctx,
                                          wo_tiles.popleft())
                        emit_attention_head(c, g, h, cur["q_sb"], ctx_sb)
                        if nxt is not None:
                            for _ in range(STEPS[h]):
                                emit_qkv_step(nxt, jt_mm)
                                jt_mm += 1
                            if h == 3:
                                emit_rope(nxt, 0)
                            if h == QPK * GPC - 1:
                                emit_rope(nxt, 1)
                        if c > 0 or h >= QPK * GPC - 2:
                            wo_issue()
                prev_ctx = ctx_sb
                if nxt is not None:
                    cur = nxt

            # drain: output projection for the final strip
            for ec in range(ECN):
                emit_wo_block(SC - 1, ec, prev_ctx, wo_tiles.popleft())
                wo_issue()
    nc.finalize()
    return nc


def _rope_tables(scale):
    inv = 1.0 / (ROPE_BASE ** (np.arange(0, D, 2, dtype=np.float64) / D))
    ang = np.arange(S, dtype=np.float64)[None, :] * inv[:, None]    # [D/2, S]
    C = np.empty((D, S), np.float32)
    Sx = np.empty((D, S), np.float32)
    C[0::2] = np.cos(ang)
    C[1::2] = np.cos(ang)
    Sx[0::2] = -np.sin(ang)
    Sx[1::2] = np.sin(ang)
    return (C * scale).astype(np.float32), (Sx * scale).astype(np.float32)


def _host_inputs(x, wqkv, wo):
    """Shard + retile inputs for the 8 cores. Core c = 4*db + t."""
    cq, sq = _rope_tables(D ** -0.5)
    ck, sk = _rope_tables(1.0)

    # causal mask tiles in scores^T layout: keep when jj + 128*r <= ii
    jj = np.arange(128)[:, None]
    ii = np.arange(CW)[None, :]
    mk = np.empty((128, SC, CW), np_bf16)
    for r in range(SC):
        mk[:, r, :] = (jj + 128 * r <= ii).astype(np_bf16)

    xts = []
    for db in range(DP):
        xT = np.ascontiguousarray(x[db].T)                 # [E, S]
        t = xT.reshape(NET, 128, SC, CW).transpose(2, 1, 0, 3)
        xts.append(np.ascontiguousarray(t.astype(np_bf16)))

    wqs, wos = [], []
    rows = (E + 2 * KVH * D) // TP
    for t in range(TP):
        wT = np.ascontiguousarray(wqkv[rows * t:rows * (t + 1)].T)   # [E, 1536]
        wq_t = wT.reshape(NET, 128, NJT, 128).transpose(2, 1, 0, 3)
        wqs.append(np.ascontiguousarray(wq_t.astype(np_bf16)))
        woT = np.ascontiguousarray(wo[:, 1024 * t:1024 * (t + 1)].T)  # [1024, E]
        wo_t = woT.reshape(FT, 128, ECN, CW).transpose(2, 1, 0, 3)
        wos.append(np.ascontiguousarray(wo_t.astype(np_bf16)))

    in_maps = []
    for c in range(NCORES):
        db, t = divmod(c, TP)
        in_maps.append({
            "xt": xts[db], "wq": wqs[t], "wo": wos[t],
            "cq": cq, "sq": sq, "ck": ck, "sk": sk,
            "mk": mk,
        })
    return in_maps


def kernel(x, wqkv, wo):
    x = np.asarray(x, np.float32)
    wqkv = np.asarray(wqkv, np.float32)
    wo = np.asarray(wo, np.float32)

    if "nc" not in _built:
        _built["nc"] = _build_nc()
    nc = _built["nc"]

    in_maps = _host_inputs(x, wqkv, wo)
    res = run_bass_kernel_spmd(nc, in_maps, core_ids=list(range(NCORES)))
    globals()["_last_results"] = res

    out = np.zeros((B, S, E), np.float32)
    for c in range(NCORES):
        db = c // TP
        o = np.asarray(res.results[c]["out"], dtype=np.float32)
        # [SC, ECN, 128, 4, CW] -> [SC, 4, 128, ECN, CW] -> [S, E]
        out[db] += o.transpose(0, 3, 2, 1, 4).reshape(S, E)
    return out


# revision 11
# speedup vs baseline: 1.2701x; 1.0172x over previous
"""Bass/Trainium2 kernel for GQA transformer block (nn_GQA_84353157694016).

v2: fully software-pipelined schedule.  Each "attention phase" for strip c
interleaves, per head: the previous strip's output-projection block, the
NEXT strip's qkv-projection j-tiles (weights prefetched one head ahead),
and the head's own score/softmax/AV work.  This removes the serial
qkv-phase <-> attention-phase transitions entirely: the PE sees a uniform
mix of dense matmul work while ACT/DVE/GpSimd run softmax underneath.

Other deltas vs v1:
  - v is transposed by the DMA xbar (dma_start_transpose) straight from the
    projection PSUM copy, freeing the PE transposes and 3 scalar copies.
  - all input loads ride sync/scalar/gpsimd queues so no engine FIFO ever
    blocks behind a transfer it doesn't need.
"""

import os
import sys

import numpy as np
import ml_dtypes

for _p in ("/opt/trn_rl_repo",):
    if _p not in sys.path and os.path.isdir(_p):
        sys.path.append(_p)

import concourse.bass as bass
import concourse.tile as tile
from concourse import bacc, mybir
from concourse.bass_utils import run_bass_kernel_spmd
from concourse.masks import make_identity


def _install_ntff_hook():
    try:
        import antenv.axon_hooks  # noqa: F401
        return
    except ImportError:
        pass
    try:
        import types
        import antenv
        mod = types.ModuleType("antenv.axon_hooks")
        mod._hook = None
        mod.set_axon_ntff_profile_hook = lambda h: setattr(mod, "_hook", h)
        mod.get_axon_ntff_profile_hook = lambda: mod._hook
        sys.modules["antenv.axon_hooks"] = mod
        antenv.axon_hooks = mod
        from trn_agent_boot.trn_boot import _ntff_profile_via_ctypes
        so = "/opt/axon/libaxon_pjrt.so"
        if os.path.exists(so):
            mod._hook = _ntff_profile_via_ctypes(so)
    except Exception:
        pass


_install_ntff_hook()

# problem constants
B, S, E = 2, 2048, 4096
H, KVH, D = 32, 8, 128
QPK = H // KVH                    # 4 q heads per kv group
ROPE_BASE = 10000.0

NCORES = 8
TP = 4                            # tensor-parallel width (heads)
DP = 2                            # data-parallel width (batch)

SC = 4                            # number of s-chunks == q strips
CW = S // SC                      # 512 chunk width
NJT = (E + 2 * KVH * D) // TP // 128   # 12 qkv row-tiles per core
NET = E // 128                    # 32 contraction tiles for qkv proj
GPC = KVH // TP                   # 2 kv groups per core
HPC = H // TP                     # 8 q heads per core
FT = HPC * D // 128               # 8 local ctx feature tiles
ECN = E // CW                     # 8 output e-chunks

# qkv j-tiles of strip c+1 whose matmuls run inside head h of strip c's
# attention; the DMA for a tile is issued one head earlier.
STEPS = (2, 1, 2, 1, 2, 1, 2, 1)

f32 = mybir.dt.float32
bf16 = mybir.dt.bfloat16
np_bf16 = ml_dtypes.bfloat16

_built = {}


def _build_nc():
    nc = bacc.Bacc("TRN2", target_bir_lowering=False)

    xt_d = nc.dram_tensor("xt", [SC, 128, NET, CW], bf16, kind="ExternalInput")
    wq_d = nc.dram_tensor("wq", [NJT, 128, NET, 128], bf16, kind="ExternalInput")
    wo_d = nc.dram_tensor("wo", [ECN, 128, FT, CW], bf16, kind="ExternalInput")
    cq_d = nc.dram_tensor("cq", [128, S], f32, kind="ExternalInput")
    sq_d = nc.dram_tensor("sq", [128, S], f32, kind="ExternalInput")
    ck_d = nc.dram_tensor("ck", [128, S], f32, kind="ExternalInput")
    sk_d = nc.dram_tensor("sk", [128, S], f32, kind="ExternalInput")
    mk_d = nc.dram_tensor("mk", [128, SC, CW], bf16, kind="ExternalInput")
    out_d = nc.dram_tensor("out", [SC, ECN, 128, CW // 128, CW], bf16,
                           kind="ExternalOutput")

    from contextlib import ExitStack
    with tile.TileContext(nc) as tc:
        with ExitStack() as _stk:
            def _pool(*a, **kw):
                return _stk.enter_context(tc.tile_pool(*a, **kw))
            constp = _pool(name="const", bufs=1)
            tabp = _pool(name="tab", bufs=2)
            xtp = _pool(name="xt", bufs=5)
            wqp = _pool(name="wq", bufs=4)
            stp = _pool(name="st", bufs=6)
            rtp = _pool(name="rt", bufs=2)
            qp = _pool(name="q", bufs=2)
            kvp = _pool(name="kv", bufs=1)
            atp = _pool(name="at", bufs=3)
            accp = _pool(name="acc", bufs=2)
            ctxp = _pool(name="ctx", bufs=2)
            wop = _pool(name="wop", bufs=3)
            obp = _pool(name="ob", bufs=2)
            vtp = _pool(name="vt", bufs=1)
            rcp = _pool(name="rc", bufs=1)
            pmm = _pool(name="pmm", bufs=2, space="PSUM")
            pqk = _pool(name="pqk", bufs=2, space="PSUM")
            pacc = _pool(name="pacc", bufs=2, space="PSUM")

            def issue_wo(ec, drain=False):
                """Prefetch the wo weight block for e-chunk ec."""
                wo_sb = wop.tile([128, FT, CW], bf16, tag="wo", name="wo_sb")
                if drain:
                    _e = (nc.sync, nc.scalar, nc.gpsimd)
                    engs = (_e[(2 * ec) % 3], _e[(2 * ec + 1) % 3])
                else:
                    engs = (nc.scalar, nc.gpsimd)
                engs[0].dma_start(out=wo_sb[:, :FT // 2, :],
                                  in_=wo_d[ec, :, :FT // 2, :])
                engs[1].dma_start(out=wo_sb[:, FT // 2:, :],
                                  in_=wo_d[ec, :, FT // 2:, :])
                return wo_sb

            def emit_wo_block(cs, ec, ctx_tiles, wo_sb):
                """Output-projection block: out[strip cs, ec] += ctx @ woT."""
                ob = obp.tile([128, CW // 128, CW], bf16, tag="ob", name="ob")
                for sti in range(CW // 128):
                    ps = pacc.tile([128, CW], f32, tag="acc", name="wo_ps")
                    for ft in range(FT):
                        nc.tensor.matmul(
                            ps,
                            lhsT=ctx_tiles[:, ft, sti * 128:(sti + 1) * 128],
                            rhs=wo_sb[:, ft, :],
                            start=(ft == 0),
                            stop=(ft == FT - 1),
                        )
                    nc.vector.tensor_copy(ob[:, sti, :], ps)
                (nc.sync, nc.gpsimd)[ec % 2].dma_start(out=out_d[cs, ec], in_=ob)

            # constants
            ones_sb = constp.tile([128, 128], bf16, tag="ones")
            nc.vector.memset(ones_sb, 1.0)
            mk_sb = constp.tile([128, SC, CW], bf16, tag="mk")

            # persistent k (transposed) and v (natural) per kv group, bf16
            k_sb = [kvp.tile([128, S], bf16, tag=f"k{g}", name=f"k{g}")
                    for g in range(GPC)]
            v_sb = [kvp.tile([128, S // 128, 128], bf16, tag=f"v{g}", name=f"v{g}")
                    for g in range(GPC)]

            def start_chunk(c):
                """Allocate per-strip tiles + issue the first input DMAs."""
                st = {"c": c, "stage": {0: [], 1: []}, "wq": {}}
                for pre in range(2):
                    w_ = wqp.tile([128, NET, 128], bf16, tag="wq", name="wq_pre")
                    nc.sync.dma_start(out=w_[:, :NET // 2, :],
                                      in_=wq_d[pre, :, :NET // 2, :])
                    (nc.scalar, nc.gpsimd)[pre].dma_start(
                        out=w_[:, NET // 2:, :],
                        in_=wq_d[pre, :, NET // 2:, :])
                    st["wq"][pre] = w_
                xh = []
                NQ = NET // 4
                for qt in range(4):
                    xh_t = xtp.tile([128, NQ, CW], bf16, tag="xt", name="xh")
                    (nc.scalar, nc.gpsimd)[qt % 2].dma_start(
                        out=xh_t,
                        in_=xt_d[c, :, qt * NQ:(qt + 1) * NQ, :],
                    )
                    xh.append(xh_t)
                st["xh"] = xh
                csl = slice(c * CW, (c + 1) * CW)
                tabs = []
                for src in (cq_d, sq_d, ck_d, sk_d):
                    t_ = tabp.tile([128, CW], f32, tag=f"tab{len(tabs)}",
                                   name="tab")
                    nc.gpsimd.dma_start(out=t_, in_=src[:, csl])
                    tabs.append(t_)
                st["tabs"] = tabs
                st["q_sb"] = qp.tile([128, HPC, CW], bf16, tag="q", name="q_sb")
                return st

            def issue_wq(st, jt):
                """Prefetch the weight tile for j-tile jt (two heads ahead)."""
                if jt >= 2 and jt < NJT:
                    w_ = wqp.tile([128, NET, 128], bf16, tag="wq", name="wq_jt")
                    e2 = (nc.scalar, nc.gpsimd)[jt % 2]
                    nc.sync.dma_start(out=w_[:, :NET // 2, :],
                                      in_=wq_d[jt, :, :NET // 2, :])
                    e2.dma_start(out=w_[:, NET // 2:, :],
                                 in_=wq_d[jt, :, NET // 2:, :])
                    st["wq"][jt] = w_

            def emit_qkv_step(st, jt):
                """Matmuls + drain for one qkv j-tile of strip st['c']."""
                c = st["c"]
                g, sub = divmod(jt, 6)
                wq_sb = st["wq"].pop(jt)
                ps = pmm.tile([128, CW], f32, tag="mm", name="qkv_ps")
                for et in range(NET):
                    nc.tensor.matmul(
                        ps,
                        lhsT=wq_sb[:, et, :],
                        rhs=st["xh"][et // (NET // 4)][:, et % (NET // 4), :],
                        start=(et == 0),
                        stop=(et == NET - 1),
                    )
                if sub == 5:
                    # v tile: cast to bf16 once, then DMA-xbar transpose into
                    # the persistent [kpos, d] v buffer (no PE transpose)
                    vt = vtp.tile([128, CW], bf16, tag="vt", name="vt")
                    nc.scalar.copy(vt, ps)
                    for u in range(CW // 128):
                        nc.sync.dma_start_transpose(
                            out=v_sb[g][:, (CW // 128) * c + u, :],
                            in_=vt[:, u * 128:(u + 1) * 128],
                        )
                else:
                    s_t = stp.tile([128, CW], f32, tag="st", name="st")
                    nc.scalar.copy(s_t, ps)
                    st["stage"][g].append(s_t)

            def emit_rope(st, g):
                """RoPE for the 4 q tiles + 1 k tile of group g."""
                c = st["c"]
                csl = slice(c * CW, (c + 1) * CW)
                cq_sb, sq_sb, ck_sb, sk_sb = st["tabs"]
                stage = st["stage"][g]
                for sub in range(QPK + 1):
                    stq = stage[sub]
                    is_q = sub < QPK
                    sw = rtp.tile([128, CW], f32, tag="sw")
                    nc.gpsimd.dma_start(out=sw[0::2, :], in_=stq[1::2, :])
                    nc.gpsimd.dma_start(out=sw[1::2, :], in_=stq[0::2, :])
                    tmp = rtp.tile([128, CW], f32, tag="rt")
                    nc.vector.tensor_mul(tmp, sw, sq_sb if is_q else sk_sb)
                    nc.vector.tensor_mul(stq, stq, cq_sb if is_q else ck_sb)
                    if is_q:
                        nc.vector.tensor_add(
                            st["q_sb"][:, QPK * g + sub, :], stq, tmp)
                    else:
                        nc.vector.tensor_add(k_sb[g][:, csl], stq, tmp)

            def emit_attention_head(c, g, h, q_sb, ctx_sb):
                """Scores + softmax + AV for one head of strip c."""
                njt2 = (CW // 128) * (c + 1)
                ctx_ps = pacc.tile([128, CW], f32, tag="acc", name="ctx_ps")
                accL = accp.tile([128, CW], bf16, tag="accL")
                accR = accp.tile([128, CW], bf16, tag="accR")
                for jp in range(njt2 // 2):
                    qk = pqk.tile([128, 2 * CW], f32, tag="qk", name="qk")
                    offs = []
                    for half in range(2):
                        j2 = 2 * jp + half
                        diag = j2 >= njt2 - (CW // 128)
                        o = 128 * (j2 - (njt2 - (CW // 128))) if diag else 0
                        offs.append((j2, o, diag))
                        nc.tensor.matmul(
                            qk[:, half * CW + o:(half + 1) * CW],
                            lhsT=k_sb[g][:, j2 * 128:(j2 + 1) * 128],
                            rhs=q_sb[:, h, o:],
                            start=True, stop=True,
                        )
                    # one exp over the whole 2-bank pair; masked (garbage)
                    # columns of diagonal tiles are never read downstream
                    at = atp.tile([128, 2 * CW], bf16, tag="at")
                    nc.scalar.activation(
                        at, qk, mybir.ActivationFunctionType.Exp
                    )
                    for half in range(2):
                        j2, o, diag = offs[half]
                        base = half * CW
                        if diag:
                            nc.vector.tensor_mul(
                                at[:, base + o:base + o + 128],
                                at[:, base + o:base + o + 128],
                                mk_sb[:, 0, :128],
                            )
                        nc.tensor.matmul(
                            ctx_ps[:, o:],
                            lhsT=v_sb[g][:, j2, :],
                            rhs=at[:, base + o:base + CW],
                            start=(j2 == 0), stop=(j2 == njt2 - 1),
                        )
                        # row-sum accumulators: two independent DVE chains
                        eng = nc.vector
                        acc_t = accL if half == 0 else accR
                        if jp == 0:
                            if o:
                                nc.gpsimd.memset(acc_t[:, :o], 0.0)
                            eng.tensor_copy(
                                acc_t[:, o:], at[:, base + o:base + CW]
                            )
                        else:
                            eng.tensor_add(
                                acc_t[:, o:], acc_t[:, o:],
                                at[:, base + o:base + CW],
                            )
                sums = pqk.tile([128, 2 * CW], f32, tag="qk", name="sums")
                nc.tensor.matmul(sums[:, :CW], lhsT=ones_sb, rhs=accL,
                                 start=True, stop=False)
                nc.tensor.matmul(sums[:, :CW], lhsT=ones_sb, rhs=accR,
                                 start=False, stop=True)
                rc = rcp.tile([128, CW], f32, tag="rc")
                nc.vector.reciprocal_approx_fast(out=rc, in_=sums[:, :CW])
                nc.vector.tensor_mul(ctx_sb[:, h, :], ctx_ps, rc)

            # ---------------- main pipelined schedule ----------------
            cur = start_chunk(0)
            nc.sync.dma_start(out=mk_sb, in_=mk_d[:])
            for jt in range(NJT):
                issue_wq(cur, jt + 2 if jt + 2 < NJT else NJT)
                emit_qkv_step(cur, jt)
                if jt == 4:
                    emit_rope(cur, 0)
            emit_rope(cur, 1)

            # wo weight blocks are prefetched two head-slots ahead across the
            # whole flat sequence (3 interleaved phases + the drain)
            from collections import deque
            wo_tiles = deque()
            wo_ctr = [0]

            def wo_issue():
                i = wo_ctr[0]
                if i >= 4 * ECN:
                    return
                wo_ctr[0] += 1
                wo_tiles.append(issue_wo(i % ECN, drain=(i >= 3 * ECN)))

            prev_ctx = None
            for c in range(SC):
                nxt = start_chunk(c + 1) if c < SC - 1 else None
                ctx_sb = ctxp.tile([128, HPC, CW], bf16, tag="ctx")
                jt_mm = 0      # next j-tile (of strip c+1) to run matmuls for
                jt_dma = [2]   # next j-tile to prefetch weights for
                cum = [0]
                for shd in STEPS:
                    cum.append(cum[-1] + shd)
                for g in range(GPC):
                    for hq in range(QPK):
                        h = QPK * g + hq
                        if nxt is not None:
                            # prefetch weights ~one head ahead, capped at the
                            # pool depth so a DMA never camps on a full queue
                            tgt = min(NJT, cum[min(h + 2, len(STEPS))],
                                      jt_mm + 4)
                            while jt_dma[0] < tgt:
                                issue_wq(nxt, jt_dma[0])
                                jt_dma[0] += 1
                        if c > 0:
                            emit_wo_block(c - 1, h, prev_ctx,
                                          wo_tiles.popleft())
                        emit_attention_head(c, g, h, cur["q_sb"], ctx_sb)
                        if nxt is not None:
                            for _ in range(STEPS[h]):
                                emit_qkv_step(nxt, jt_mm)
                                jt_mm += 1
                            if h == 2:
                                emit_rope(nxt, 0)
                            if h == QPK * GPC - 2:
                                emit_rope(nxt, 1)
                        if c > 0 or h >= QPK * GPC - 2:
                            wo_issue()
                prev_ctx = ctx_sb
                if nxt is not None:
                    cur = nxt

            # drain: output projection for the final strip
            for ec in range(ECN):
                emit_wo_block(SC - 1, ec, prev_ctx, wo_tiles.popleft())
                wo_issue()
    nc.finalize()
    return nc


def _rope_tables(scale):
    inv = 1.0 / (ROPE_BASE ** (np.arange(0, D, 2, dtype=np.float64) / D))
    ang = np.arange(S, dtype=np.float64)[None, :] * inv[:, None]    # [D/2, S]
    C = np.empty((D, S), np.float32)
    Sx = np.empty((D, S), np.float32)
    C[0::2] = np.cos(ang)
    C[1::2] = np.cos(ang)
    Sx[0::2] = -np.sin(ang)
    Sx[1::2] = np.sin(ang)
    return (C * scale).astype(np.float32), (Sx * scale).astype(np.float32)


def _host_inputs(x, wqkv, wo):
    """Shard + retile inputs for the 8 cores. Core c = 4*db + t."""
    cq, sq = _rope_tables(D ** -0.5)
    ck, sk = _rope_tables(1.0)

    # causal mask tiles in scores^T layout: keep when jj + 128*r <= ii
    jj = np.arange(128)[:, None]
    ii = np.arange(CW)[None, :]
    mk = np.empty((128, SC, CW), np_bf16)
    for r in range(SC):
        mk[:, r, :] = (jj + 128 * r <= ii).astype(np_bf16)

    xts = []
    for db in range(DP):
        xT = np.ascontiguousarray(x[db].T)                 # [E, S]
        t = xT.reshape(NET, 128, SC, CW).transpose(2, 1, 0, 3)
        xts.append(np.ascontiguousarray(t.astype(np_bf16)))

    wqs, wos = [], []
    rows = (E + 2 * KVH * D) // TP
    for t in range(TP):
        wT = np.ascontiguousarray(wqkv[rows * t:rows * (t + 1)].T)   # [E, 1536]
        wq_t = wT.reshape(NET, 128, NJT, 128).transpose(2, 1, 0, 3)
        wqs.append(np.ascontiguousarray(wq_t.astype(np_bf16)))
        woT = np.ascontiguousarray(wo[:, 1024 * t:1024 * (t + 1)].T)  # [1024, E]
        wo_t = woT.reshape(FT, 128, ECN, CW).transpose(2, 1, 0, 3)
        wos.append(np.ascontiguousarray(wo_t.astype(np_bf16)))

    in_maps = []
    for c in range(NCORES):
        db, t = divmod(c, TP)
        in_maps.append({
            "xt": xts[db], "wq": wqs[t], "wo": wos[t],
            "cq": cq, "sq": sq, "ck": ck, "sk": sk,
            "mk": mk,
        })
    return in_maps


def kernel(x, wqkv, wo):
    x = np.asarray(x, np.float32)
    wqkv = np.asarray(wqkv, np.float32)
    wo = np.asarray(wo, np.float32)

    if "nc" not in _built:
        _built["nc"] = _build_nc()
    nc = _built["nc"]

    in_maps = _host_inputs(x, wqkv, wo)
    res = run_bass_kernel_spmd(nc, in_maps, core_ids=list(range(NCORES)))
    globals()["_last_results"] = res

    out = np.zeros((B, S, E), np.float32)
    for c in range(NCORES):
        db = c // TP
        o = np.asarray(res.results[c]["out"], dtype=np.float32)
        # [SC, ECN, 128, 4, CW] -> [SC, 4, 128, ECN, CW] -> [S, E]
        out[db] += o.transpose(0, 3, 2, 1, 4).reshape(S, E)
    return out
